# revision 62
# baseline (speedup 1.0000x reference)
"""Batched Viterbi decode (BiLSTM-CRF forward) on 8 Trainium2 NeuronCores.

Problem: feats [B=256, T=2048, K=64] f32, transitions [K, K] f32.
  fv_0 = init (-1e4 everywhere, 0 at start_tag)
  per step t: scores[b,n,p] = fv[b,p] + trans[n,p]
              bp[t][b,n]   = argmax_p scores   (first max wins, = jnp.argmax)
              fv[b,n]      = max_p scores + feats[b,t,n]
  terminal   = fv_last + trans[stop_tag]; path_score = max; backtrace via bp.

Sharding: pure data-parallel on batch. 32 sequences per core.

Device layout (per core):
  SBUF partitions = 128 = (g in 0..3) * 32 + b(local 0..31); tag n = g*16 + ns.
  fv tile [128, 16]:        fv[(g,b), ns] = fv_val[b, g*16+ns]
  Per step:
    PE  : builds scores[(g,b), (p, ns)] = fv_val[b, p] + T[g*16+ns, p] directly
          in PSUM: per 512-col bank, one start=True matmul writes the T rows
          (exact 1.0*x products), then one-hot matmuls accumulate the fv
          broadcast (single fp32 accumulate = one rounding, bit-identical to
          the reference's fv + T elementwise add; verified exact on HW).
    DVE : m = grouped reduce_max over p (transposed AP view)    [128, 16]
          fv = m + feat_t                                       (tensor add)
          lt = scores < m (bcast)                               -> SBUF
          reversed tensor_tensor_scan state = lt*(state+1): after visiting
          position q (scanning high->low) the state is the distance to the
          nearest match at-or-above q, so at each group's p=0 it equals the
          group's FIRST argmax index (ties break like jnp.argmax).
          bpw[.., ns] = scan value at (ns, p=0)                 (strided copy)
  bpw streamed to HBM as f32; host converts to int backpointers and runs the
  (trivial, O(B*T)) pointer-chase backtrace plus terminal argmax in numpy f32,
  which is bit-exact to the reference's jax-on-cpu f32 arithmetic.

Container quirks worked around here: walrus rejects >1 sync-wait per
instruction (fixed by _legalize_sync_json), rejects SWDGE-DMA and custom-DVE
encodings entirely ("ISA wrong length"), and For_i back-edge drains need >2
waits — so the program is fully unrolled (no hardware loop) with all DMAs on
one HWDGE sem proc.
"""

import os
import sys

import numpy as np

if "/opt/trn_rl_repo" not in sys.path:
    sys.path.insert(0, "/opt/trn_rl_repo")

B, T, K = 256, 2048, 64
NCORES = 8
B_LOC = B // NCORES  # 32
G = 4                # partition groups
NS = K // G          # 16 tags per group
NEG_INF = -10000.0
CHUNK = 128          # timesteps per hardware-loop iteration

_NC_CACHE: dict = {}
TRACE = False          # set True (e.g. from test.py) to capture an NTFF profile
LAST_RESULTS = None    # BassKernelResults of the most recent device run


def _legalize_sync_json(bir_json_bytes: bytes) -> bytes:
    """This container's walrus rejects any instruction whose sync_info has >1
    wait ("Too many sync wait commands"). Split excess waits onto preceding
    NoOp instructions on the same engine (sems are monotonic, so sequential
    waits are equivalent to a combined wait)."""
    import json as _json

    d = _json.loads(bir_json_bytes)
    ctr = 0
    for fn in d["functions"]:
        for blk in fn["blocks"]:
            out = []
            for inst in blk["instructions"]:
                si = inst.get("sync_info")
                waits = (si or {}).get("on_wait") or []
                if len(waits) > 1:
                    for w in waits[:-1]:
                        ctr += 1
                        out.append({
                            "name": f"{inst['name']}-lw{ctr}",
                            "opcode": "NoOp",
                            "engine": inst["engine"],
                            "ins": [],
                            "outs": [],
                            "sync_info": {"on_wait": [w], "on_update": []},
                            "debug": inst.get("debug"),
                        })
                    si["on_wait"] = waits[-1:]
                out.append(inst)
            blk["instructions"] = out
    return _json.dumps(d).encode()


def _legalize_nc(nc):
    from concourse import mybir

    bj = mybir.module_to_json_bytes(nc.m)
    nc.m = mybir.module_from_json_string(_legalize_sync_json(bj).decode())
    return nc


def _bp_mark_ref(in0, in1, c0, c1, c2):
    """CoreSim reference: out = -Idx where in0>=in1 else -FLT_MAX."""
    p = in0.shape[0]
    x = np.asarray(in0, np.float32).reshape(p, -1)
    y = np.asarray(in1, np.float32).reshape(p, -1)
    idx = np.arange(x.shape[1], dtype=np.float32)
    return np.where(x >= y, -idx, np.float32(-3.4028234663852886e38))


def _register_bp_mark():
    """Fused backpointer-mark op: one DVE pass instead of is_ge + weight-mult.
    out[q,k] = -k if scores[q,k] >= m_bcast[q,k] else -FLT_MAX; a grouped
    reduce_max over each 64-wide block then yields -(first argmax position)."""
    import concourse.dve_ops as dops
    from concourse.dve_spec import Spec, Src0, Src1, Zero, MaxNeg, Idx, select, lower
    from concourse.dve_spec import _has_src1 as has_src1
    from concourse.dve_uop import DveOpSpec

    name = "BP_MARK_ANT"
    if name in dops._SUB_OPCODE_FOR_NAME:
        return next(op for op in dops.OPS if op.name == name)
    spec = Spec(body=select(Src0 >= Src1, Zero - Idx, MaxNeg), reference=_bp_mark_ref)
    opcode = dops._CUSTOM_DVE_ROW_BASE + len(dops.OPS)
    assert opcode < 0x20
    shas = {}
    for ver in ("v3", "v4"):
        ds = DveOpSpec(name=name, opcode=opcode, uops=lower(spec, ver=ver),
                       rd1_en=has_src1(spec))
        shas[ver] = ds.sha(ver)
    op = dops.DveOp(name, spec, subdim=False, uops_sha=shas)
    dops.OPS.append(op)
    dops.CUSTOM_DVE_SPECS[name] = spec
    dops._SUB_OPCODE_FOR_NAME[name] = opcode
    return op


def build_nc(t_total: int = T, chunk: int = CHUNK, legalize: bool = True):
    """Build the per-core Bass program (same NEFF for all 8 cores)."""
    import concourse.bass as bass
    import concourse.mybir as mybir
    import concourse.tile as tile

    fp32 = mybir.dt.float32
    add = mybir.AluOpType.add
    amax = mybir.AluOpType.max
    is_ge = mybir.AluOpType.is_ge
    is_lt = mybir.AluOpType.is_lt
    is_gt = mybir.AluOpType.is_gt
    sub = mybir.AluOpType.subtract
    mult = mybir.AluOpType.mult
    AX = mybir.AxisListType.X

    nchunk = t_total // chunk
    assert nchunk * chunk == t_total

    import concourse.tile_sem_assignment as tsa

    bp_mark = _register_bp_mark()
    nc = bass.Bass("TRN2", debug=False)

    # One input blob per core: [feats (t_total*NS) | t_rep | lhs_t | wt | fv0]
    # packed along the free dim so a SINGLE preload DMA (one HWDGE queue sem)
    # brings everything into SBUF — the loop back-edge drain has a ~4-slot
    # sync-wait cap in walrus codegen, so DMA queue sems are precious.
    CW = NS * K + G * 128 + K + NS + NS * K + 128
    FW = t_total * NS
    blob_d = nc.dram_tensor("blob", [128, FW + CW], fp32, kind="ExternalInput")
    bpw_d = nc.dram_tensor("bpw", [128, t_total, NS], fp32, kind="ExternalOutput")
    fvl_d = nc.dram_tensor("fv_last", [128, NS], fp32, kind="ExternalOutput")

    # All HWDGE DMAs share ONE vector-clock proc (and sem): the loop back-edge
    # drain has only 3 sync-wait slots in walrus CoreV3 codegen and must cover
    # PE + DVE + DMA. (gpsimd/SWDGE DMAs are avoided entirely: this walrus
    # build rejects their encoding with "ISA wrong length".)
    _old_hw_sems = tsa.NUM_HWDGE_SEMS
    tsa.NUM_HWDGE_SEMS = 1
    with tile.TileContext(nc) as tc:
        with (
            tc.tile_pool(name="const", bufs=1) as cpool,
            tc.tile_pool(name="feat", bufs=1) as fpool,
            tc.tile_pool(name="featc", bufs=2) as fcpool,
            tc.tile_pool(name="score", bufs=2) as spool,
            tc.tile_pool(name="m", bufs=4) as mpool,
            tc.tile_pool(name="scan", bufs=2) as snpool,
            tc.tile_pool(name="dd", bufs=2) as dpool,
            tc.tile_pool(name="lt", bufs=2) as ltpool,
            tc.tile_pool(name="bp", bufs=2) as bpool,
            tc.tile_pool(name="psum", bufs=3, space="PSUM") as ppool,
        ):
            blob = fpool.tile([128, FW + CW], fp32)
            fv = cpool.tile([128, NS], fp32)
            nc.sync.dma_start(blob[:], blob_d.ap())
            feats_sb = blob[:, 0:FW]
            o0, o1, o2, o3 = FW, FW + NS * K, FW + NS * K + G * 128, FW + NS * K + G * 128 + K
            trep = blob[:, o0:o1]
            lhst = blob[:, o1:o2]
            wt = blob[:, o2:o3]
            o4 = o3 + NS              # T_re rows 0-3: [4, NS*K]
            o5 = o4 + NS * K          # lhsT5 rows 0-3: [4, 128]
            t_re = blob[0:4, o4:o4 + NS * K]
            lhst5 = blob[0:4, o5:o5 + 128]
            # fv's writer is the DVE, merging its wait into the DVE clock
            nc.vector.tensor_copy(fv[:], blob[:, o3:o3 + NS])

            trep3 = trep.rearrange("q (s n) -> q s n", n=K)

            for ci in range(nchunk):
                bpb = bpool.tile([128, chunk * NS], fp32)
                for c in range(chunk):
                    t = ci * chunk + c
                    # PE builds scores[(g,b), (ns,p)] = fv[b,p] + T[g*16+ns, p]
                    # directly in PSUM: 4 one-hot matmuls broadcast fv (exact
                    # 1.0*x products), then one accumulating matmul adds the
                    # T rows (single fp32 accumulate = one rounding, matching
                    # the reference's fv + T elementwise add).
                    # PSUM free layout is (p, ns) so every matmul output is a
                    # contiguous block: MM_g covers cols [g*256,(g+1)*256) with
                    # fv[b, g*16+nsp] (exact 1.0*x products), then two
                    # accumulating matmuls add the T rows per 512-col PSUM bank
                    # (single fp32 accumulate = one rounding, matching the
                    # reference's fv + T elementwise add). DVE reads it back
                    # through a transposed AP view with p innermost.
                    ps = ppool.tile([128, NS * K], fp32)
                    fv_b = fv[:].unsqueeze(2).to_broadcast([128, NS, NS])
                    for h in range(2):
                        # T rows first: one start=True matmul owns the bank
                        nc.tensor.matmul(
                            ps[:, h * 512:(h + 1) * 512],
                            lhst5,
                            t_re[:, h * 512:(h + 1) * 512],
                            start=True,
                            stop=False,
                            skip_group_check=True,
                        )
                        for g in (2 * h, 2 * h + 1):
                            nc.tensor.matmul(
                                ps[:, g * NS * NS:(g + 1) * NS * NS],
                                lhst[:, g * 128:(g + 1) * 128],
                                fv_b,
                                start=False,
                                stop=(g % 2 == 1),
                                skip_group_check=True,
                            )
                    ps3 = ps[:].rearrange("q (p s) -> q s p", s=NS)
                    m = mpool.tile([128, NS], fp32)
                    nc.vector.tensor_reduce(m[:], ps3, axis=AX, op=amax)
                    # fv update first so PE can start step t+1 immediately
                    nc.vector.tensor_tensor(
                        fv[:], m[:], feats_sb[:, t * NS:(t + 1) * NS], op=add
                    )
                    # backpointer extraction: lt = (scores < m) in place, then a
                    # REVERSED-stream scan state = lt*(state+1). After visiting
                    # position q (scanning high->low), state = distance to the
                    # nearest match at-or-above q, so at each group's p=0 the
                    # value is exactly the group's FIRST argmax index (ties
                    # break like jnp.argmax). Group crossings are washed out by
                    # the reset at each group's own max.
                    # ACT re-layouts scores PSUM(p,ns) -> SBUF (ns,p)
                    # (overlaps the DVE reduce); GPSIMD then computes
                    # d = m - scores (d == 0 exactly at maxima, > 0 elsewhere,
                    # IEEE-monotone so never rounds to 0 for a non-max); DVE
                    # turns it into the 0/1 mask with a single-src
                    # tensor_scalar is_gt (2x perf mode) and runs the scan.
                    m_b = m[:].unsqueeze(2).to_broadcast([128, NS, K])
                    lt = ltpool.tile([128, NS * K], fp32)
                    nc.vector.tensor_tensor(
                        lt[:].rearrange("q (s n) -> q s n", n=K),
                        ps3, m_b, op=is_lt,
                    )
                    scn = snpool.tile([128, NS * K], fp32)
                    nc.vector.tensor_tensor_scan(
                        scn[:, ::-1], lt[:, ::-1], lt[:, ::-1], 0.0,
                        op0=mult, op1=add,
                    )
                    nc.vector.tensor_copy(
                        bpb[:, c * NS:(c + 1) * NS],
                        scn[:].rearrange("q (s n) -> q s n", n=K)[:, :, 0],
                    )
                nc.sync.dma_start(
                    bpw_d.ap()[:, ci * chunk:(ci + 1) * chunk, :],
                    bpb[:].rearrange("q (c s) -> q c s", s=NS),
                )
            nc.sync.dma_start(fvl_d.ap(), fv[:])

    tsa.NUM_HWDGE_SEMS = _old_hw_sems
    nc.finalize()
    if legalize:
        _legalize_nc(nc)
    return nc


def _host_constants(transitions: np.ndarray, start_tag: int):
    """Static per-core input tensors (identical on every core)."""
    trans = np.asarray(transitions, np.float32)
    # t_rep[(g,b), ns*K + p] = trans[g*16+ns, p]
    trep = np.empty((128, NS * K), np.float32)
    for g in range(G):
        blk = trans[g * NS:(g + 1) * NS, :].reshape(NS * K)
        trep[g * B_LOC:(g + 1) * B_LOC, :] = blk[None, :]
    # lhs_t[(gc,bc), g*128 + (go,bo)] = (gc==g) & (bc==bo)
    lhst = np.zeros((128, G * 128), np.float32)
    for g in range(G):
        for bo in range(B_LOC):
            for go in range(G):
                lhst[g * B_LOC + bo, g * 128 + go * B_LOC + bo] = 1.0
    wt = np.broadcast_to(
        (K - np.arange(K, dtype=np.float32))[None, :], (128, K)
    ).copy()
    fv0 = np.full((128, NS), NEG_INF, np.float32)
    g0, ns0 = divmod(int(start_tag), NS)
    fv0[g0 * B_LOC:(g0 + 1) * B_LOC, ns0] = 0.0
    # T_re rows 0-3: T_re[g', ns*K+p] = trans[g'*16+ns, p]
    t_re = np.zeros((128, NS * K), np.float32)
    for gp in range(G):
        blk = trans[gp * NS:(gp + 1) * NS, :]          # [ns, p]
        t_re[gp, :] = np.ascontiguousarray(blk.T).reshape(NS * K)  # (p, ns)
    # lhsT5 rows 0-3: lhst5[g', (g*32+b)] = (g' == g)
    lhst5 = np.zeros((128, 128), np.float32)
    for gp in range(G):
        lhst5[gp, gp * B_LOC:(gp + 1) * B_LOC] = 1.0
    return np.ascontiguousarray(
        np.concatenate([trep, lhst, wt, fv0, t_re, lhst5], axis=1))


def _feats_to_dev(feats_core: np.ndarray) -> np.ndarray:
    """[32, T, 64] -> [128, T, 16] with partition (g,b), tag n = g*16+ns."""
    t_total = feats_core.shape[1]
    f = feats_core.reshape(B_LOC, t_total, G, NS)       # [b, t, g, ns]
    return np.ascontiguousarray(f.transpose(2, 0, 1, 3).reshape(128, t_total, NS))


def _postprocess(bpw_cores, fvl_cores, transitions, stop_tag, t_total):
    """bpw [core][128, T, 16] f32, fvl [core][128,16] -> (path_score, best_path)."""
    trans = np.asarray(transitions, np.float32)
    bn = len(bpw_cores) * B_LOC
    # backpointers: bp[t, b_global, n]
    bp = np.empty((t_total, bn, K), np.int32)
    fvl = np.empty((bn, K), np.float32)
    for ci, (bpw, fl) in enumerate(zip(bpw_cores, fvl_cores)):
        # bpw[(g,b), t, ns] = first argmax index p within the group
        p4 = (np.rint(bpw.astype(np.float64)).astype(np.int32) % K)  # [128, T, 16]
        p4 = p4.reshape(G, B_LOC, t_total, NS)                 # [g, b, t, ns]
        bp[:, ci * B_LOC:(ci + 1) * B_LOC, :] = (
            p4.transpose(2, 1, 0, 3).reshape(t_total, B_LOC, K)
        )
        fvl[ci * B_LOC:(ci + 1) * B_LOC, :] = (
            fl.reshape(G, B_LOC, NS).transpose(1, 0, 2).reshape(B_LOC, K)
        )
    terminal = fvl + trans[int(stop_tag)][None, :]             # f32, exact
    best_last = np.argmax(terminal, axis=1).astype(np.int32)
    path_score = terminal[np.arange(bn), best_last]
    path = np.empty((t_total, bn), np.int32)
    tag = best_last
    bidx = np.arange(bn)
    for t in range(t_total - 1, -1, -1):
        path[t] = tag
        tag = bp[t, bidx, tag]
    return path_score.astype(np.float32), np.ascontiguousarray(path.T).astype(np.int32)


def _run_device(nc, feats, transitions, start_tag):
    from concourse.bass_utils import run_bass_kernel_spmd

    consts = _host_constants(transitions, start_tag)
    t_total = feats.shape[1]
    in_maps = []
    for c in range(NCORES):
        fd = _feats_to_dev(feats[c * B_LOC:(c + 1) * B_LOC]).reshape(128, t_total * NS)
        in_maps.append({"blob": np.ascontiguousarray(np.concatenate([fd, consts], axis=1))})
    try:
        res = run_bass_kernel_spmd(nc, in_maps, list(range(NCORES)), trace=TRACE)
    except ModuleNotFoundError:
        # NTFF profile hook unavailable in this container — run untraced
        res = run_bass_kernel_spmd(nc, in_maps, list(range(NCORES)), trace=False)
    global LAST_RESULTS
    LAST_RESULTS = res
    outs = res.results
    bpw_cores = [np.asarray(o["bpw"]) for o in outs]
    fvl_cores = [np.asarray(o["fv_last"]) for o in outs]
    return bpw_cores, fvl_cores


def kernel(feats, transitions, start_tag, stop_tag):
    feats = np.asarray(feats, np.float32)
    transitions = np.asarray(transitions, np.float32)
    t_total = feats.shape[1]
    key = (t_total, CHUNK)
    if key not in _NC_CACHE:
        _NC_CACHE[key] = build_nc(t_total, CHUNK)
    nc = _NC_CACHE[key]
    bpw_cores, fvl_cores = _run_device(nc, feats, transitions, int(start_tag))
    return _postprocess(bpw_cores, fvl_cores, transitions, int(stop_tag), t_total)


# revision 63
# speedup vs baseline: 1.0055x; 1.0055x over previous
"""Batched Viterbi decode (BiLSTM-CRF forward) on 8 Trainium2 NeuronCores.

Problem: feats [B=256, T=2048, K=64] f32, transitions [K, K] f32.
  fv_0 = init (-1e4 everywhere, 0 at start_tag)
  per step t: scores[b,n,p] = fv[b,p] + trans[n,p]
              bp[t][b,n]   = argmax_p scores   (first max wins, = jnp.argmax)
              fv[b,n]      = max_p scores + feats[b,t,n]
  terminal   = fv_last + trans[stop_tag]; path_score = max; backtrace via bp.

Sharding: pure data-parallel on batch. 32 sequences per core.

Device layout (per core):
  SBUF partitions = 128 = (g in 0..3) * 32 + b(local 0..31); tag n = g*16 + ns.
  fv tile [128, 16]:        fv[(g,b), ns] = fv_val[b, g*16+ns]
  Per step:
    PE  : builds scores[(g,b), (p, ns)] = fv_val[b, p] + T[g*16+ns, p] directly
          in PSUM: per 512-col bank, one start=True matmul writes the T rows
          (exact 1.0*x products), then one-hot matmuls accumulate the fv
          broadcast (single fp32 accumulate = one rounding, bit-identical to
          the reference's fv + T elementwise add; verified exact on HW).
    DVE : m = grouped reduce_max over p (transposed AP view)    [128, 16]
          fv = m + feat_t                                       (tensor add)
          lt = scores < m (bcast)                               -> SBUF
          reversed tensor_tensor_scan state = lt*(state+1): after visiting
          position q (scanning high->low) the state is the distance to the
          nearest match at-or-above q, so at each group's p=0 it equals the
          group's FIRST argmax index (ties break like jnp.argmax).
          bpw[.., ns] = scan value at (ns, p=0)                 (strided copy)
  bpw streamed to HBM as f32; host converts to int backpointers and runs the
  (trivial, O(B*T)) pointer-chase backtrace plus terminal argmax in numpy f32,
  which is bit-exact to the reference's jax-on-cpu f32 arithmetic.

Container quirks worked around here: walrus rejects >1 sync-wait per
instruction (fixed by _legalize_sync_json), rejects SWDGE-DMA and custom-DVE
encodings entirely ("ISA wrong length"), and For_i back-edge drains need >2
waits — so the program is fully unrolled (no hardware loop) with all DMAs on
one HWDGE sem proc.
"""

import os
import sys

import numpy as np

if "/opt/trn_rl_repo" not in sys.path:
    sys.path.insert(0, "/opt/trn_rl_repo")

B, T, K = 256, 2048, 64
NCORES = 8
B_LOC = B // NCORES  # 32
G = 4                # partition groups
NS = K // G          # 16 tags per group
NEG_INF = -10000.0
CHUNK = 128          # timesteps per hardware-loop iteration

_NC_CACHE: dict = {}
TRACE = False          # set True (e.g. from test.py) to capture an NTFF profile
LAST_RESULTS = None    # BassKernelResults of the most recent device run


def _legalize_sync_json(bir_json_bytes: bytes) -> bytes:
    """This container's walrus rejects any instruction whose sync_info has >1
    wait ("Too many sync wait commands"). Split excess waits onto preceding
    NoOp instructions on the same engine (sems are monotonic, so sequential
    waits are equivalent to a combined wait)."""
    import json as _json

    d = _json.loads(bir_json_bytes)
    ctr = 0
    for fn in d["functions"]:
        for blk in fn["blocks"]:
            out = []
            for inst in blk["instructions"]:
                si = inst.get("sync_info")
                waits = (si or {}).get("on_wait") or []
                if len(waits) > 1:
                    for w in waits[:-1]:
                        ctr += 1
                        out.append({
                            "name": f"{inst['name']}-lw{ctr}",
                            "opcode": "NoOp",
                            "engine": inst["engine"],
                            "ins": [],
                            "outs": [],
                            "sync_info": {"on_wait": [w], "on_update": []},
                            "debug": inst.get("debug"),
                        })
                    si["on_wait"] = waits[-1:]
                out.append(inst)
            blk["instructions"] = out
    return _json.dumps(d).encode()


def _legalize_nc(nc):
    from concourse import mybir

    bj = mybir.module_to_json_bytes(nc.m)
    nc.m = mybir.module_from_json_string(_legalize_sync_json(bj).decode())
    return nc


def _bp_mark_ref(in0, in1, c0, c1, c2):
    """CoreSim reference: out = -Idx where in0>=in1 else -FLT_MAX."""
    p = in0.shape[0]
    x = np.asarray(in0, np.float32).reshape(p, -1)
    y = np.asarray(in1, np.float32).reshape(p, -1)
    idx = np.arange(x.shape[1], dtype=np.float32)
    return np.where(x >= y, -idx, np.float32(-3.4028234663852886e38))


def _register_bp_mark():
    """Fused backpointer-mark op: one DVE pass instead of is_ge + weight-mult.
    out[q,k] = -k if scores[q,k] >= m_bcast[q,k] else -FLT_MAX; a grouped
    reduce_max over each 64-wide block then yields -(first argmax position)."""
    import concourse.dve_ops as dops
    from concourse.dve_spec import Spec, Src0, Src1, Zero, MaxNeg, Idx, select, lower
    from concourse.dve_spec import _has_src1 as has_src1
    from concourse.dve_uop import DveOpSpec

    name = "BP_MARK_ANT"
    if name in dops._SUB_OPCODE_FOR_NAME:
        return next(op for op in dops.OPS if op.name == name)
    spec = Spec(body=select(Src0 >= Src1, Zero - Idx, MaxNeg), reference=_bp_mark_ref)
    opcode = dops._CUSTOM_DVE_ROW_BASE + len(dops.OPS)
    assert opcode < 0x20
    shas = {}
    for ver in ("v3", "v4"):
        ds = DveOpSpec(name=name, opcode=opcode, uops=lower(spec, ver=ver),
                       rd1_en=has_src1(spec))
        shas[ver] = ds.sha(ver)
    op = dops.DveOp(name, spec, subdim=False, uops_sha=shas)
    dops.OPS.append(op)
    dops.CUSTOM_DVE_SPECS[name] = spec
    dops._SUB_OPCODE_FOR_NAME[name] = opcode
    return op


def build_nc(t_total: int = T, chunk: int = CHUNK, legalize: bool = True):
    """Build the per-core Bass program (same NEFF for all 8 cores)."""
    import concourse.bass as bass
    import concourse.mybir as mybir
    import concourse.tile as tile

    fp32 = mybir.dt.float32
    add = mybir.AluOpType.add
    amax = mybir.AluOpType.max
    is_ge = mybir.AluOpType.is_ge
    is_lt = mybir.AluOpType.is_lt
    is_gt = mybir.AluOpType.is_gt
    sub = mybir.AluOpType.subtract
    mult = mybir.AluOpType.mult
    AX = mybir.AxisListType.X

    nchunk = t_total // chunk
    assert nchunk * chunk == t_total

    import concourse.tile_sem_assignment as tsa

    bp_mark = _register_bp_mark()
    nc = bass.Bass("TRN2", debug=False)

    # One input blob per core: [feats (t_total*NS) | t_rep | lhs_t | wt | fv0]
    # packed along the free dim so a SINGLE preload DMA (one HWDGE queue sem)
    # brings everything into SBUF — the loop back-edge drain has a ~4-slot
    # sync-wait cap in walrus codegen, so DMA queue sems are precious.
    CW = NS * K + G * 128 + K + NS + NS * K + 128
    FW = t_total * NS
    blob_d = nc.dram_tensor("blob", [128, FW + CW], fp32, kind="ExternalInput")
    bpw_d = nc.dram_tensor("bpw", [128, t_total, NS], fp32, kind="ExternalOutput")
    fvl_d = nc.dram_tensor("fv_last", [128, NS], fp32, kind="ExternalOutput")

    # All HWDGE DMAs share ONE vector-clock proc (and sem): the loop back-edge
    # drain has only 3 sync-wait slots in walrus CoreV3 codegen and must cover
    # PE + DVE + DMA. (gpsimd/SWDGE DMAs are avoided entirely: this walrus
    # build rejects their encoding with "ISA wrong length".)
    _old_hw_sems = tsa.NUM_HWDGE_SEMS
    tsa.NUM_HWDGE_SEMS = 1
    with tile.TileContext(nc) as tc:
        with (
            tc.tile_pool(name="const", bufs=1) as cpool,
            tc.tile_pool(name="feat", bufs=1) as fpool,
            tc.tile_pool(name="featc", bufs=2) as fcpool,
            tc.tile_pool(name="score", bufs=2) as spool,
            tc.tile_pool(name="m", bufs=4) as mpool,
            tc.tile_pool(name="scan", bufs=2) as snpool,
            tc.tile_pool(name="dd", bufs=2) as dpool,
            tc.tile_pool(name="lt", bufs=2) as ltpool,
            tc.tile_pool(name="bp", bufs=2) as bpool,
            tc.tile_pool(name="psum", bufs=3, space="PSUM") as ppool,
        ):
            # Split the preload so step 0 doesn't wait on the full 16.6MB:
            # consts + first chunk land via two small DMAs, the feats tail
            # streams behind the first chunk's compute (separate tiles give
            # the dependency tracker separate wait targets).
            F1 = chunk * NS
            consts_sb = cpool.tile([128, CW], fp32)
            feats1 = fpool.tile([128, F1], fp32)
            feats2 = fpool.tile([128, FW - F1], fp32)
            fv = cpool.tile([128, NS], fp32)
            nc.sync.dma_start(consts_sb[:], blob_d.ap()[:, FW:FW + CW])
            nc.sync.dma_start(feats1[:], blob_d.ap()[:, 0:F1])
            nc.sync.dma_start(feats2[:], blob_d.ap()[:, F1:FW])
            o0, o1, o2, o3 = 0, NS * K, NS * K + G * 128, NS * K + G * 128 + K
            trep = consts_sb[:, o0:o1]
            lhst = consts_sb[:, o1:o2]
            wt = consts_sb[:, o2:o3]
            o4 = o3 + NS              # T_re rows 0-3: [4, NS*K]
            o5 = o4 + NS * K          # lhsT5 rows 0-3: [4, 128]
            t_re = consts_sb[0:4, o4:o4 + NS * K]
            lhst5 = consts_sb[0:4, o5:o5 + 128]
            # fv's writer is the DVE, merging its wait into the DVE clock
            nc.vector.tensor_copy(fv[:], consts_sb[:, o3:o3 + NS])

            trep3 = trep.rearrange("q (s n) -> q s n", n=K)

            for ci in range(nchunk):
                bpb = bpool.tile([128, chunk * NS], fp32)
                for c in range(chunk):
                    t = ci * chunk + c
                    # PE builds scores[(g,b), (ns,p)] = fv[b,p] + T[g*16+ns, p]
                    # directly in PSUM: 4 one-hot matmuls broadcast fv (exact
                    # 1.0*x products), then one accumulating matmul adds the
                    # T rows (single fp32 accumulate = one rounding, matching
                    # the reference's fv + T elementwise add).
                    # PSUM free layout is (p, ns) so every matmul output is a
                    # contiguous block: MM_g covers cols [g*256,(g+1)*256) with
                    # fv[b, g*16+nsp] (exact 1.0*x products), then two
                    # accumulating matmuls add the T rows per 512-col PSUM bank
                    # (single fp32 accumulate = one rounding, matching the
                    # reference's fv + T elementwise add). DVE reads it back
                    # through a transposed AP view with p innermost.
                    ps = ppool.tile([128, NS * K], fp32)
                    fv_b = fv[:].unsqueeze(2).to_broadcast([128, NS, NS])
                    for h in range(2):
                        # T rows first: one start=True matmul owns the bank
                        nc.tensor.matmul(
                            ps[:, h * 512:(h + 1) * 512],
                            lhst5,
                            t_re[:, h * 512:(h + 1) * 512],
                            start=True,
                            stop=False,
                            skip_group_check=True,
                        )
                        for g in (2 * h, 2 * h + 1):
                            nc.tensor.matmul(
                                ps[:, g * NS * NS:(g + 1) * NS * NS],
                                lhst[:, g * 128:(g + 1) * 128],
                                fv_b,
                                start=False,
                                stop=(g % 2 == 1),
                                skip_group_check=True,
                            )
                    ps3 = ps[:].rearrange("q (p s) -> q s p", s=NS)
                    m = mpool.tile([128, NS], fp32)
                    nc.vector.tensor_reduce(m[:], ps3, axis=AX, op=amax)
                    # fv update first so PE can start step t+1 immediately
                    fsrc = (feats1[:, t * NS:(t + 1) * NS] if t < chunk
                            else feats2[:, (t - chunk) * NS:(t - chunk + 1) * NS])
                    nc.vector.tensor_tensor(fv[:], m[:], fsrc, op=add)
                    # backpointer extraction: lt = (scores < m) in place, then a
                    # REVERSED-stream scan state = lt*(state+1). After visiting
                    # position q (scanning high->low), state = distance to the
                    # nearest match at-or-above q, so at each group's p=0 the
                    # value is exactly the group's FIRST argmax index (ties
                    # break like jnp.argmax). Group crossings are washed out by
                    # the reset at each group's own max.
                    # ACT re-layouts scores PSUM(p,ns) -> SBUF (ns,p)
                    # (overlaps the DVE reduce); GPSIMD then computes
                    # d = m - scores (d == 0 exactly at maxima, > 0 elsewhere,
                    # IEEE-monotone so never rounds to 0 for a non-max); DVE
                    # turns it into the 0/1 mask with a single-src
                    # tensor_scalar is_gt (2x perf mode) and runs the scan.
                    m_b = m[:].unsqueeze(2).to_broadcast([128, NS, K])
                    lt = ltpool.tile([128, NS * K], fp32)
                    nc.vector.tensor_tensor(
                        lt[:].rearrange("q (s n) -> q s n", n=K),
                        ps3, m_b, op=is_lt,
                    )
                    scn = snpool.tile([128, NS * K], fp32)
                    nc.vector.tensor_tensor_scan(
                        scn[:, ::-1], lt[:, ::-1], lt[:, ::-1], 0.0,
                        op0=mult, op1=add,
                    )
                    nc.vector.tensor_copy(
                        bpb[:, c * NS:(c + 1) * NS],
                        scn[:].rearrange("q (s n) -> q s n", n=K)[:, :, 0],
                    )
                nc.sync.dma_start(
                    bpw_d.ap()[:, ci * chunk:(ci + 1) * chunk, :],
                    bpb[:].rearrange("q (c s) -> q c s", s=NS),
                )
            nc.sync.dma_start(fvl_d.ap(), fv[:])

    tsa.NUM_HWDGE_SEMS = _old_hw_sems
    nc.finalize()
    if legalize:
        _legalize_nc(nc)
    return nc


def _host_constants(transitions: np.ndarray, start_tag: int):
    """Static per-core input tensors (identical on every core)."""
    trans = np.asarray(transitions, np.float32)
    # t_rep[(g,b), ns*K + p] = trans[g*16+ns, p]
    trep = np.empty((128, NS * K), np.float32)
    for g in range(G):
        blk = trans[g * NS:(g + 1) * NS, :].reshape(NS * K)
        trep[g * B_LOC:(g + 1) * B_LOC, :] = blk[None, :]
    # lhs_t[(gc,bc), g*128 + (go,bo)] = (gc==g) & (bc==bo)
    lhst = np.zeros((128, G * 128), np.float32)
    for g in range(G):
        for bo in range(B_LOC):
            for go in range(G):
                lhst[g * B_LOC + bo, g * 128 + go * B_LOC + bo] = 1.0
    wt = np.broadcast_to(
        (K - np.arange(K, dtype=np.float32))[None, :], (128, K)
    ).copy()
    fv0 = np.full((128, NS), NEG_INF, np.float32)
    g0, ns0 = divmod(int(start_tag), NS)
    fv0[g0 * B_LOC:(g0 + 1) * B_LOC, ns0] = 0.0
    # T_re rows 0-3: T_re[g', ns*K+p] = trans[g'*16+ns, p]
    t_re = np.zeros((128, NS * K), np.float32)
    for gp in range(G):
        blk = trans[gp * NS:(gp + 1) * NS, :]          # [ns, p]
        t_re[gp, :] = np.ascontiguousarray(blk.T).reshape(NS * K)  # (p, ns)
    # lhsT5 rows 0-3: lhst5[g', (g*32+b)] = (g' == g)
    lhst5 = np.zeros((128, 128), np.float32)
    for gp in range(G):
        lhst5[gp, gp * B_LOC:(gp + 1) * B_LOC] = 1.0
    return np.ascontiguousarray(
        np.concatenate([trep, lhst, wt, fv0, t_re, lhst5], axis=1))


def _feats_to_dev(feats_core: np.ndarray) -> np.ndarray:
    """[32, T, 64] -> [128, T, 16] with partition (g,b), tag n = g*16+ns."""
    t_total = feats_core.shape[1]
    f = feats_core.reshape(B_LOC, t_total, G, NS)       # [b, t, g, ns]
    return np.ascontiguousarray(f.transpose(2, 0, 1, 3).reshape(128, t_total, NS))


def _postprocess(bpw_cores, fvl_cores, transitions, stop_tag, t_total):
    """bpw [core][128, T, 16] f32, fvl [core][128,16] -> (path_score, best_path)."""
    trans = np.asarray(transitions, np.float32)
    bn = len(bpw_cores) * B_LOC
    # backpointers: bp[t, b_global, n]
    bp = np.empty((t_total, bn, K), np.int32)
    fvl = np.empty((bn, K), np.float32)
    for ci, (bpw, fl) in enumerate(zip(bpw_cores, fvl_cores)):
        # bpw[(g,b), t, ns] = first argmax index p within the group
        p4 = (np.rint(bpw.astype(np.float64)).astype(np.int32) % K)  # [128, T, 16]
        p4 = p4.reshape(G, B_LOC, t_total, NS)                 # [g, b, t, ns]
        bp[:, ci * B_LOC:(ci + 1) * B_LOC, :] = (
            p4.transpose(2, 1, 0, 3).reshape(t_total, B_LOC, K)
        )
        fvl[ci * B_LOC:(ci + 1) * B_LOC, :] = (
            fl.reshape(G, B_LOC, NS).transpose(1, 0, 2).reshape(B_LOC, K)
        )
    terminal = fvl + trans[int(stop_tag)][None, :]             # f32, exact
    best_last = np.argmax(terminal, axis=1).astype(np.int32)
    path_score = terminal[np.arange(bn), best_last]
    path = np.empty((t_total, bn), np.int32)
    tag = best_last
    bidx = np.arange(bn)
    for t in range(t_total - 1, -1, -1):
        path[t] = tag
        tag = bp[t, bidx, tag]
    return path_score.astype(np.float32), np.ascontiguousarray(path.T).astype(np.int32)


def _run_device(nc, feats, transitions, start_tag):
    from concourse.bass_utils import run_bass_kernel_spmd

    consts = _host_constants(transitions, start_tag)
    t_total = feats.shape[1]
    in_maps = []
    for c in range(NCORES):
        fd = _feats_to_dev(feats[c * B_LOC:(c + 1) * B_LOC]).reshape(128, t_total * NS)
        in_maps.append({"blob": np.ascontiguousarray(np.concatenate([fd, consts], axis=1))})
    try:
        res = run_bass_kernel_spmd(nc, in_maps, list(range(NCORES)), trace=TRACE)
    except ModuleNotFoundError:
        # NTFF profile hook unavailable in this container — run untraced
        res = run_bass_kernel_spmd(nc, in_maps, list(range(NCORES)), trace=False)
    global LAST_RESULTS
    LAST_RESULTS = res
    outs = res.results
    bpw_cores = [np.asarray(o["bpw"]) for o in outs]
    fvl_cores = [np.asarray(o["fv_last"]) for o in outs]
    return bpw_cores, fvl_cores


def kernel(feats, transitions, start_tag, stop_tag):
    feats = np.asarray(feats, np.float32)
    transitions = np.asarray(transitions, np.float32)
    t_total = feats.shape[1]
    key = (t_total, CHUNK)
    if key not in _NC_CACHE:
        _NC_CACHE[key] = build_nc(t_total, CHUNK)
    nc = _NC_CACHE[key]
    bpw_cores, fvl_cores = _run_device(nc, feats, transitions, int(start_tag))
    return _postprocess(bpw_cores, fvl_cores, transitions, int(stop_tag), t_total)
